# revision 1
# baseline (speedup 1.0000x reference)
"""Trainium2 Bass kernel for nn_ConvAttention_34600256537137.

Math notes (validated against the reference to ~3e-6 rel err):
  qkv = 1x1conv(x, w1)+b1 -> Q,K,V;  score = conv5x5(Q_s)+conv5x5(K_t)+b2;
  attn = softmax_t(score);  out = einsum(attn, V).
  Softmax over t is shift-invariant, so the Q-half of the score (constant in
  t), b2, and the K-path bias all cancel.  The computation collapses to:
    weff[ci,dy,dx] = sum_c w1K[c,ci] * w2K[c,dy,dx]        (host, tiny)
    sK[b,t,h,w]    = conv5x5_reflect(x[b,:,:,:,t], weff)
    e = exp(sK);  den = sum_t e
    out[b,o,h,w,s] = (sum_{ci,t} w1V[o,ci] * e * x) / den + b1V[o]
  (s-independent; normalization folded to the end; bias added on host)

Sharding: 8 cores = (b in {0,1}) x (4 chunks of 8 rows of H).  All reflect
padding and layout transforms are precomputed host-side so every core runs an
identical program on its own slices.

Perf structure (v3):
  - DMAs are spread round-robin over all 5 engine queues (a single queue
    serializes descriptors at ~600ns each).
  - slab arrives as 6 row-pair tiles so conv matmuls start as rows land.
  - score conv: T[tap,pos] = weff^T @ slab streamed once (24 half-row
    matmuls); PSUM->SBUF copies write T as (tap, t, row, w); DRAM bounce +
    25 per-tap gathers build R[(t,h), tap, w]; DVE reduces taps on 128 lanes.
  - softmax denominator via indicator-matmul on PE (no partition reduce);
    normalization happens on the final PSUM->SBUF read.
"""

import sys

if "/opt/trn_rl_repo" not in sys.path:
    sys.path.insert(0, "/opt/trn_rl_repo")

import numpy as np

B, C, H, W, S = 2, 64, 32, 32, 16
KS, PAD = 5, 2
NCORES = 8
ROWS = H // 4            # output rows per core
SLAB_R = ROWS + 2 * PAD  # 12
SLAB_W = W + 2 * PAD     # 36
NTAP = KS * KS           # 25
NPOS = SLAB_R * SLAB_W * S  # 6912 slab positions
HW = ROWS * W            # 256 output positions

_MODULE = None


def _build_module():
    import concourse.bacc as bacc
    import concourse.bass as bass
    import concourse.tile as tile
    from concourse import mybir

    f32 = mybir.dt.float32
    AF = mybir.ActivationFunctionType
    ALU = mybir.AluOpType
    nc = bacc.Bacc("TRN2", target_bir_lowering=False, debug=False, num_devices=NCORES)

    slab_d = nc.dram_tensor("slab", [C, SLAB_R, SLAB_W, S], f32, kind="ExternalInput")
    xt_d = nc.dram_tensor("xt", [128, 8, HW], f32, kind="ExternalInput")
    weff_d = nc.dram_tensor("weff", [C, NTAP], f32, kind="ExternalInput")
    w1vr_d = nc.dram_tensor("w1vr", [128, 8, C], f32, kind="ExternalInput")
    hsel_d = nc.dram_tensor("hsel", [128, ROWS], f32, kind="ExternalInput")
    o_d = nc.dram_tensor("o", [C, S, HW], f32, kind="ExternalOutput")

    # scratch DRAM for partition-crossing rearrangements
    td_d = nc.dram_tensor("td", [NTAP, S, SLAB_R, SLAB_W], f32)   # T, t-major
    ed_d = nc.dram_tensor("ed", [S, ROWS, W], f32)                # exp(sK), t-major
    dend_d = nc.dram_tensor("dend", [ROWS * W], f32)              # 1/den, flat hw

    engs = None
    _rr = [0]

    def dma(out, in_):
        e = engs[_rr[0] % len(engs)]
        _rr[0] += 1
        e.dma_start(out, in_)

    with tile.TileContext(nc) as tc:
        engs = [nc.sync, nc.scalar, nc.gpsimd]
        with tc.tile_pool(name="sb", bufs=1) as sb, tc.tile_pool(
            name="ps", bufs=6, space="PSUM"
        ) as ps, tc.tile_pool(name="pso", bufs=1, space="PSUM") as pso:
            # --- loads: weff tiny on gpsimd; slab pairs lead sync/scalar ---
            s_weff = sb.tile([C, NTAP], f32)
            nc.gpsimd.dma_start(s_weff, weff_d.ap())
            slab_t = []
            for rp in range(6):
                st = sb.tile([C, 2, SLAB_W, S], f32, tag=f"slab{rp}")
                slab_t.append(st)
                (nc.sync, nc.scalar, nc.gpsimd)[rp % 3].dma_start(
                    st, slab_d.ap()[:, 2 * rp : 2 * rp + 2, :, :]
                )
            s_hsel = sb.tile([128, ROWS], f32)
            nc.gpsimd.dma_start(s_hsel, hsel_d.ap())
            s_xt = sb.tile([128, 8, HW], f32)
            nc.sync.dma_start(s_xt, xt_d.ap())
            s_w1vr = sb.tile([128, 8, C], f32)
            nc.scalar.dma_start(s_w1vr, w1vr_d.ap())

            # --- phase 1: T[tap, (row, w, t)] = weff^T @ slab, half-row chunks
            # s_T2 holds T transposed to (tap, t, row, w): w contiguous.
            s_T2 = sb.tile([NTAP, S, SLAB_R, SLAB_W], f32)
            HREST = SLAB_W // 2  # 18
            for hr in range(SLAB_R * 2):
                row, half = divmod(hr, 2)
                p_t = ps.tile([NTAP, HREST, S], f32, tag="pt")
                nc.tensor.matmul(
                    p_t,
                    s_weff,
                    slab_t[row // 2][:, row % 2, half * HREST : (half + 1) * HREST, :],
                    start=True,
                    stop=True,
                )
                # copy PSUM -> s_T2[(tap), t, row, w-half] (strided write)
                eng = nc.vector if hr % 2 == 0 else nc.scalar
                if eng is nc.vector:
                    eng.tensor_copy(
                        s_T2[:, :, row, half * HREST : (half + 1) * HREST],
                        p_t.transpose([0, 2, 1]),
                    )
                else:
                    eng.copy(
                        s_T2[:, :, row, half * HREST : (half + 1) * HREST],
                        p_t.transpose([0, 2, 1]),
                    )

            # --- T to DRAM (contiguous both sides), 6 row-pair chunks so
            # gathers can pipeline behind the conv copies ---
            for ci in range(6):
                e = (nc.sync, nc.scalar, nc.gpsimd)[ci % 3]
                e.dma_start(
                    td_d.ap()[:, :, 2 * ci : 2 * ci + 2, :],
                    s_T2[:, :, 2 * ci : 2 * ci + 2, :],
                )

            # --- 25 per-tap gathers into R[(t,h) 128p, tap, w] ---
            s_R = sb.tile([128, NTAP, W], f32)
            for k in range(NTAP):
                dyi, dxi = divmod(k, KS)
                src = bass.AP(
                    tensor=td_d.ap().tensor,
                    offset=k * NPOS + dyi * SLAB_W + dxi,
                    ap=[[SLAB_R * SLAB_W, S], [SLAB_W, ROWS], [1, W]],
                )
                dma(s_R[:, k, :], src)

            # --- tap reduce on 128 lanes (strided view puts tap innermost) ---
            s_sk = sb.tile([128, W], f32)  # [(t,h), w]
            nc.vector.tensor_reduce(
                s_sk, s_R.transpose([0, 2, 1]), axis=mybir.AxisListType.X, op=ALU.add
            )

            # --- e = exp(sK) in [(t,h), w]; den via indicator-matmul on PE ---
            s_e = sb.tile([128, W], f32)
            nc.scalar.activation(s_e, s_sk, AF.Exp)
            p_den = pso.tile([ROWS, W], f32, tag="den")
            nc.tensor.matmul(p_den, s_hsel, s_e, start=True, stop=True)
            s_rcp = sb.tile([ROWS, W], f32)
            nc.vector.reciprocal(s_rcp, p_den)
            nc.scalar.dma_start(dend_d.ap(), s_rcp)
            s_rcpb = sb.tile([C, HW], f32)
            nc.scalar.dma_start(
                s_rcpb,
                bass.AP(tensor=dend_d.ap().tensor, offset=0, ap=[[0, C], [1, HW]]),
            )

            # --- bounce e to [t, hw] and read back as [(ci8,t), hw] ---
            # (s_e partitions iterate (t, h) so the flat [t, h, w] layout of
            # ed_d matches the source order directly)
            nc.sync.dma_start(ed_d.ap(), s_e)
            s_eb = sb.tile([128, HW], f32)
            for g in range(8):
                src = bass.AP(
                    tensor=ed_d.ap().tensor,
                    offset=0,
                    ap=[[ROWS * W, S], [1, ROWS * W]],  # (t, hw)
                )
                (nc.sync if g % 2 == 0 else nc.scalar).dma_start(
                    s_eb[g * S : (g + 1) * S, :], src
                )

            # --- V path: xattn = x_t * e; contract (ci,t) on PE ---
            s_xa = sb.tile([128, 8, HW], f32)
            nc.vector.tensor_tensor(
                s_xa,
                s_xt,
                s_eb.unsqueeze(1).broadcast_to((128, 8, HW)),
                op=ALU.mult,
            )
            p_o = pso.tile([C, HW], f32, tag="out")
            for g in range(8):
                nc.tensor.matmul(
                    p_o,
                    s_w1vr[:, g, :],
                    s_xa[:, g, :],
                    start=(g == 0),
                    stop=(g == 7),
                )
            # normalize on the PSUM->SBUF read
            s_o = sb.tile([C, HW], f32)
            nc.vector.tensor_tensor(s_o, p_o, s_rcpb, op=ALU.mult)
            bounds = [0, 22, 43, C]
            for ci, e in enumerate((nc.sync, nc.scalar, nc.gpsimd)):
                a, b = bounds[ci], bounds[ci + 1]
                e.dma_start(
                    o_d.ap()[a:b],
                    s_o[a:b].unsqueeze(1).broadcast_to((b - a, S, HW)),
                )

    nc.compile()
    return nc


def _get_module():
    global _MODULE
    if _MODULE is None:
        _MODULE = _build_module()
    return _MODULE


def make_host_inputs(x, w1, b1, w2, b2):
    """Host-side precompute: folded weights + per-core reflect-padded slices."""
    x = np.ascontiguousarray(np.asarray(x, np.float32))
    w1 = np.asarray(w1, np.float32)
    w2 = np.asarray(w2, np.float32)

    w1K = w1[C : 2 * C, :, 0, 0]          # [c, ci]
    w2K = w2[0, C : 2 * C]                # [c, 5, 5]
    weff = np.ascontiguousarray(
        np.einsum("ci,cyx->iyx", w1K, w2K).reshape(C, NTAP)
    )
    w1V = w1[2 * C :, :, 0, 0]            # [co, ci]

    # w1vr[(ci8,t), g, co] = w1V[co, 8g+ci8]
    tmp = w1V.T.reshape(8, 8, C)                      # (g, ci8, co)
    w1vr = np.ascontiguousarray(
        np.broadcast_to(tmp[:, :, None, :], (8, 8, S, C))
        .transpose(1, 2, 0, 3)
        .reshape(128, 8, C)
    )

    # hsel[(t,h), m] = 1 if h == m  (partition index = t*ROWS + h)
    hsel = np.zeros((128, ROWS), np.float32)
    for t in range(S):
        for h in range(ROWS):
            hsel[t * ROWS + h, h] = 1.0

    in_maps = []
    for core in range(NCORES):
        b, hc = divmod(core, 4)
        h0 = ROWS * hc
        xp = np.pad(x[b], ((0, 0), (PAD, PAD), (PAD, PAD), (0, 0)), mode="reflect")
        slab = np.ascontiguousarray(xp[:, h0 : h0 + SLAB_R, :, :])
        xs = x[b][:, h0 : h0 + ROWS, :, :]            # [ci, h, w, t]
        xt = np.ascontiguousarray(
            xs.reshape(8, 8, ROWS, W, S)
            .transpose(1, 4, 0, 2, 3)
            .reshape(128, 8, HW)
        )
        in_maps.append(
            {"slab": slab, "xt": xt, "weff": weff, "w1vr": w1vr, "hsel": hsel}
        )
    return in_maps


def assemble_output(results, b1):
    b1V = np.asarray(b1, np.float32)[2 * C :]
    out = np.empty((B, C, H, W, S), np.float32)
    for core in range(NCORES):
        b, hc = divmod(core, 4)
        h0 = ROWS * hc
        o = results[core]["o"].reshape(C, S, ROWS, W).transpose(0, 2, 3, 1)
        out[b, :, h0 : h0 + ROWS, :, :] = o
    out += b1V[None, :, None, None, None]
    return out


def kernel(x, w1, b1, w2, b2):
    from concourse.bass_utils import run_bass_kernel_spmd

    nc = _get_module()
    in_maps = make_host_inputs(x, w1, b1, w2, b2)
    res = run_bass_kernel_spmd(nc, in_maps, core_ids=list(range(NCORES)))
    return assemble_output(res.results, b1)



# revision 6
# speedup vs baseline: 1.8018x; 1.8018x over previous
"""Trainium2 Bass kernel for nn_ConvAttention_34600256537137.

Math notes (validated against the reference):
  qkv = 1x1conv(x, w1)+b1 -> Q,K,V;  score = conv5x5(Q_s)+conv5x5(K_t)+b2;
  attn = softmax_t(score);  out = einsum(attn, V).
  Softmax over t is shift-invariant, so the Q-half of the score (constant in
  t), b2, and the K-path bias all cancel.  The computation collapses to:
    weff[ci,dy,dx] = sum_c w1K[c,ci] * w2K[c,dy,dx]        (host, tiny)
    sK[b,t,h,w]    = conv5x5_reflect(x[b,:,:,:,t], weff)
    e = exp(sK);  den = sum_t e
    out[b,o,h,w,s] = (sum_{ci,t} w1V[o,ci] * e * x) / den + b1V[o]
  (s-independent; normalization + S-broadcast + bias done on host)

Sharding: 8 cores = (b in {0,1}) x (4 chunks of 8 rows of H).

Perf structure (v4):
  - bf16 end-to-end on device (PSUM accumulates fp32); 4x PE rate vs fp32.
  - phase 1 (score conv): T[tap, pos] = weff^T @ slab in 16 contiguous
    432-col matmuls; slab free layout is (row~, t, w~) so PSUM->SBUF copies
    are plain contiguous casts.
  - T bounced to DRAM in [tap, row~, t, w~] layout: the (row,t) partition
    pair merges to a single stride-36 dim, dy folds into the DMA offset and
    dx becomes a stride-(6912+1) dim -> the 25 shift-gathers collapse to
    5 DMAs (one per dy), dest R[(row,t), tap, w].
  - softmax: DVE tap-reduce on 128 lanes, exp on Act, denominator via
    indicator-matmul, reciprocal on DVE.
  - e and 1/den broadcasts: DRAM bounce, strided re-reads.
  - V path: xattn = x_t * e_b (DVE, 2 chunks); contract (ci,t) on PE in 8
    bf16 matmuls; normalization folded into the PSUM->SBUF read.
  - output is [C, HW] only (S-broadcast on host): 64KB instead of 1MB.
"""

import sys

if "/opt/trn_rl_repo" not in sys.path:
    sys.path.insert(0, "/opt/trn_rl_repo")

import numpy as np
import ml_dtypes

BF16 = ml_dtypes.bfloat16

B, C, H, W, S = 2, 64, 32, 32, 16
KS, PAD = 5, 2
NCORES = 8
ROWS = H // 4            # output rows per core
SLAB_R = ROWS + 2 * PAD  # 12
SLAB_W = W + 2 * PAD     # 36
NTAP = KS * KS           # 25
NPOS = SLAB_R * SLAB_W * S  # 6912 slab positions per tap
HW = ROWS * W            # 256 output positions
RT = SLAB_W * S          # 576 = one row~'s (t, w~) block
CH = 432                 # matmul chunk (free cols); 6912 = 16 * 432
NCH = NPOS // CH         # 16

_MODULE = None


def _build_module():
    import concourse.bacc as bacc
    import concourse.bass as bass
    import concourse.tile as tile
    from concourse import mybir

    f32 = mybir.dt.float32
    bf16 = mybir.dt.bfloat16
    AF = mybir.ActivationFunctionType
    ALU = mybir.AluOpType
    nc = bacc.Bacc("TRN2", target_bir_lowering=False, debug=False, num_devices=NCORES)

    # slab free layout per channel partition: (row~, t, w~) flat = 6912
    slab_d = nc.dram_tensor("slab", [C, SLAB_R, S, SLAB_W], bf16, kind="ExternalInput")
    xt_d = nc.dram_tensor("xt", [128, 8, HW], bf16, kind="ExternalInput")
    weff_d = nc.dram_tensor("weff", [C, NTAP], bf16, kind="ExternalInput")
    w1vr_d = nc.dram_tensor("w1vr", [128, 8, C], bf16, kind="ExternalInput")
    hsel_d = nc.dram_tensor("hsel", [128, ROWS], bf16, kind="ExternalInput")
    o_d = nc.dram_tensor("o", [C, HW], f32, kind="ExternalOutput")

    # scratch DRAM for partition-crossing rearrangements
    td_d = nc.dram_tensor("td", [NTAP, SLAB_R, S, SLAB_W], bf16)  # T, tap-major
    ed_d = nc.dram_tensor("ed", [ROWS, S, W], bf16)               # e, (row,t,w)
    dend_d = nc.dram_tensor("dend", [ROWS, W], bf16)              # 1/den

    with tile.TileContext(nc) as tc:
        with tc.tile_pool(name="sb", bufs=1) as sb, tc.tile_pool(
            name="ps", bufs=6, space="PSUM"
        ) as ps, tc.tile_pool(name="pso", bufs=1, space="PSUM") as pso:
            # --- loads: weff first (needed by every matmul), then slab pairs
            s_weff = sb.tile([C, NTAP], bf16)
            nc.scalar.dma_start(s_weff, weff_d.ap())
            s_hsel = sb.tile([128, ROWS], bf16)
            nc.gpsimd.dma_start(s_hsel, hsel_d.ap())
            s_slab = sb.tile([C, SLAB_R, S, SLAB_W], bf16)
            for rp in range(6):
                (nc.sync, nc.scalar, nc.gpsimd)[rp % 3].dma_start(
                    s_slab[:, 2 * rp : 2 * rp + 2, :, :],
                    slab_d.ap()[:, 2 * rp : 2 * rp + 2, :, :],
                )
            s_xt = sb.tile([128, 8, HW], bf16)
            nc.sync.dma_start(s_xt[:, 0:4, :], xt_d.ap()[:, 0:4, :])
            nc.scalar.dma_start(s_xt[:, 4:8, :], xt_d.ap()[:, 4:8, :])
            s_w1vr = sb.tile([128, 8, C], bf16)
            nc.gpsimd.dma_start(s_w1vr, w1vr_d.ap())

            # --- phase 1: T[tap, (row~, t, w~)] = weff^T @ slab, 432-col chunks
            s_T2 = sb.tile([NTAP, SLAB_R, S, SLAB_W], bf16)
            slab_flat = s_slab[:].rearrange("c a b d -> c (a b d)")
            t2_flat = s_T2[:].rearrange("k a b d -> k (a b d)")
            copy_engs = (nc.vector, nc.scalar)
            for ci in range(NCH):
                p_t = ps.tile([NTAP, CH], f32, tag="pt")
                nc.tensor.matmul(
                    p_t,
                    s_weff,
                    slab_flat[:, ci * CH : (ci + 1) * CH],
                    start=True,
                    stop=True,
                )
                eng = copy_engs[ci % 2]
                if eng is nc.vector:
                    eng.tensor_copy(t2_flat[:, ci * CH : (ci + 1) * CH], p_t)
                else:
                    eng.copy(t2_flat[:, ci * CH : (ci + 1) * CH], p_t)

            # --- T to DRAM (contiguous both sides), 6 row-pair chunks so
            # the dy-gathers can pipeline behind the conv copies ---
            for ci in range(6):
                e = (nc.sync, nc.scalar, nc.gpsimd)[ci % 3]
                e.dma_start(
                    td_d.ap()[:, 2 * ci : 2 * ci + 2, :, :],
                    s_T2[:, 2 * ci : 2 * ci + 2, :, :],
                )

            # --- 5 dy-gathers into R[(row,t) 128p, tap=(dy,dx), w] ---
            # td addr of elem (tap=(5dy+dx), t, row+dy, dx+w)
            #   = dy*(5*NPOS + RT) + dx*(NPOS + 1) + 36*(16*row + t) + w
            s_R = sb.tile([128, NTAP, W], bf16)
            for dy in range(KS):
                src = bass.AP(
                    tensor=td_d.ap().tensor,
                    offset=dy * (KS * NPOS + RT),
                    ap=[[SLAB_W, 128], [NPOS + 1, KS], [1, W]],
                )
                (nc.sync, nc.scalar, nc.gpsimd)[dy % 3].dma_start(
                    s_R[:, dy * KS : (dy + 1) * KS, :], src
                )

            # --- tap reduce on 128 lanes (strided view puts tap innermost) ---
            s_sk = sb.tile([128, W], f32)  # [(row,t), w]
            nc.vector.tensor_reduce(
                s_sk, s_R.transpose([0, 2, 1]), axis=mybir.AxisListType.X, op=ALU.add
            )

            # --- e = exp(sK) in [(row,t), w]; den via indicator-matmul on PE ---
            s_e = sb.tile([128, W], bf16)
            nc.scalar.activation(s_e, s_sk, AF.Exp)
            p_den = pso.tile([ROWS, W], f32, tag="den")
            nc.tensor.matmul(p_den, s_hsel, s_e, start=True, stop=True)
            s_rcp = sb.tile([ROWS, W], bf16)
            with nc.allow_low_precision(reason="1/den fits bf16; tol is 2e-2"):
                nc.vector.reciprocal(s_rcp, p_den)
            # bounce 1/den to DRAM, read back broadcast over C partitions
            nc.scalar.dma_start(dend_d.ap(), s_rcp)
            s_rcpb = sb.tile([C, HW], bf16)
            nc.scalar.dma_start(
                s_rcpb,
                bass.AP(tensor=dend_d.ap().tensor, offset=0, ap=[[0, C], [1, HW]]),
            )

            # --- bounce e to (row, t, w) and read back as [(ci8,t), (row,w)] ---
            # (s_e partitions iterate (row, t) so the flat layout matches)
            nc.sync.dma_start(ed_d.ap(), s_e)
            s_eb = sb.tile([128, HW], bf16)
            for g in range(8):
                src = bass.AP(
                    tensor=ed_d.ap().tensor,
                    offset=0,
                    ap=[[W, S], [S * W, ROWS], [1, W]],  # (t, row, w)
                )
                (nc.sync, nc.gpsimd)[g % 2].dma_start(
                    s_eb[g * S : (g + 1) * S, :], src
                )

            # --- V path: xattn = x_t * e; contract (ci,t) on PE ---
            s_xa = sb.tile([128, 8, HW], bf16)
            p_o = pso.tile([C, HW], f32, tag="out")
            for half in range(2):
                g0, g1 = 4 * half, 4 * half + 4
                nc.vector.tensor_tensor(
                    s_xa[:, g0:g1, :],
                    s_xt[:, g0:g1, :],
                    s_eb.unsqueeze(1).broadcast_to((128, 4, HW)),
                    op=ALU.mult,
                )
                for g in range(g0, g1):
                    nc.tensor.matmul(
                        p_o,
                        s_w1vr[:, g, :],
                        s_xa[:, g, :],
                        start=(g == 0),
                        stop=(g == 7),
                    )
            # normalize on the PSUM->SBUF read
            s_o = sb.tile([C, HW], f32)
            nc.vector.tensor_tensor(s_o, p_o, s_rcpb, op=ALU.mult)
            nc.sync.dma_start(o_d.ap()[0:32], s_o[0:32])
            nc.scalar.dma_start(o_d.ap()[32:64], s_o[32:64])

    nc.compile()
    return nc


def _get_module():
    global _MODULE
    if _MODULE is None:
        _MODULE = _build_module()
    return _MODULE


def make_host_inputs(x, w1, b1, w2, b2):
    """Host-side precompute: folded weights + per-core reflect-padded slices."""
    x = np.ascontiguousarray(np.asarray(x, np.float32))
    w1 = np.asarray(w1, np.float32)
    w2 = np.asarray(w2, np.float32)

    w1K = w1[C : 2 * C, :, 0, 0]          # [c, ci]
    w2K = w2[0, C : 2 * C]                # [c, 5, 5]
    weff = np.ascontiguousarray(
        np.einsum("ci,cyx->iyx", w1K, w2K).reshape(C, NTAP)
    ).astype(BF16)
    w1V = w1[2 * C :, :, 0, 0]            # [co, ci]

    # w1vr[(ci8,t), g, co] = w1V[co, 8g+ci8]
    tmp = w1V.T.reshape(8, 8, C)                      # (g, ci8, co)
    w1vr = np.ascontiguousarray(
        np.broadcast_to(tmp[:, :, None, :], (8, 8, S, C))
        .transpose(1, 2, 0, 3)
        .reshape(128, 8, C)
    ).astype(BF16)

    # hsel[(row,t), m] = 1 if row == m  (partition index = row*S + t)
    hsel = np.zeros((128, ROWS), np.float32)
    for r in range(ROWS):
        hsel[r * S : (r + 1) * S, r] = 1.0
    hsel = hsel.astype(BF16)

    in_maps = []
    for core in range(NCORES):
        b, hc = divmod(core, 4)
        h0 = ROWS * hc
        xp = np.pad(x[b], ((0, 0), (PAD, PAD), (PAD, PAD), (0, 0)), mode="reflect")
        # slab[c, row~, t, w~]
        slab = np.ascontiguousarray(
            xp[:, h0 : h0 + SLAB_R, :, :].transpose(0, 1, 3, 2)
        ).astype(BF16)
        xs = x[b][:, h0 : h0 + ROWS, :, :]            # [ci, h, w, t]
        xt = np.ascontiguousarray(
            xs.reshape(8, 8, ROWS, W, S)
            .transpose(1, 4, 0, 2, 3)
            .reshape(128, 8, HW)
        ).astype(BF16)
        in_maps.append(
            {"slab": slab, "xt": xt, "weff": weff, "w1vr": w1vr, "hsel": hsel}
        )
    return in_maps


def assemble_output(results, b1):
    b1V = np.asarray(b1, np.float32)[2 * C :]
    out = np.empty((B, C, H, W, S), np.float32)
    for core in range(NCORES):
        b, hc = divmod(core, 4)
        h0 = ROWS * hc
        o = results[core]["o"].reshape(C, ROWS, W, 1)
        out[b, :, h0 : h0 + ROWS, :, :] = o
    out += b1V[None, :, None, None, None]
    return out


def kernel(x, w1, b1, w2, b2):
    from concourse.bass_utils import run_bass_kernel_spmd

    nc = _get_module()
    in_maps = make_host_inputs(x, w1, b1, w2, b2)
    res = run_bass_kernel_spmd(nc, in_maps, core_ids=list(range(NCORES)))
    return assemble_output(res.results, b1)
